# revision 27
# baseline (speedup 1.0000x reference)
"""AdaptiveOutlierLoss on 8 TRN2 NeuronCores.

loss = mean_b relu(margin - min_c poincare_dist(z_b, proto_c))

Strategy (data-parallel over B, prototypes replicated):
  With inv_c = 1/(1 - |p_c|^2), invx_b = 1/(1 - |z_b|^2), TensorE computes
      q[b,c] = (|z_b|^2 + |p_c|^2 - 2 z.p) inv_c
             = [-2 z_b; x2_b; 1] . [p_c inv_c; inv_c; |p_c|^2 inv_c]
  (K = D + 2 = 514, fp16 operands, fp32 PSUM accumulate). dist is a
  monotone transform of q for fixed b, so min_c dist = transform(min_c q):
      arg = max(1 + 2 max(min_c q, 0) invx_b, 1 + EPS)
      dist = arccosh(arg) = ln(arg + sqrt(arg^2 - 1))
  VectorE min-reduces each [128, 2048] PSUM block; the arccosh/relu/sum
  epilogue runs on a single [128, 32] tile. Each core handles 4096 rows;
  the host sums the 8 per-core partials (gather of a sum-sharded scalar).

  Startup is DMA-bandwidth-bound, so only the transposed operands move
  (12 MB/core): the row norms x2/y2 are computed on-chip from the same
  staged chunks via elementwise Square + a ones-column matmul reduction
  over the partition axis, which also lands them directly in matmul
  column order (aug rows and the [128, 32] epilogue layout need no
  cross-partition shuffling beyond two tiny transposes).
"""

import math
import os
import sys

for _p in ("/opt/trn_rl_repo", "/root/.axon_site/_ro/trn_rl_repo"):
    if os.path.isdir(_p) and _p not in sys.path:
        sys.path.append(_p)

import numpy as np
from concourse import bacc, mybir, tile
from concourse.bass_utils import run_bass_kernel_spmd
from concourse.masks import make_identity

P = 128
D = 512
C = 2048
B = 32768
NCORES = 8
BL = B // NCORES  # 4096 rows per core
KC = D // P  # 4 contraction chunks
MT = BL // P  # 32 output row tiles
NT = C // 512  # 4 psum banks of c per row tile
EPS = 1e-7
LN2 = math.log(2.0)

MM_DT = mybir.dt.float16
F32 = mybir.dt.float32
AF = mybir.ActivationFunctionType
ALU = mybir.AluOpType
AX = mybir.AxisListType

_NC_CACHE = {}


def _build_nc():
    nc = bacc.Bacc("TRN2", target_bir_lowering=False, debug=False, num_devices=NCORES)
    zt_e = nc.declare_dram_parameter("zt", [D, BL], F32, isOutput=False)
    pt_e = nc.declare_dram_parameter("pt", [D, C], F32, isOutput=False)
    mg_e = nc.declare_dram_parameter("margin", [P, 1], F32, isOutput=False)
    out_e = nc.declare_dram_parameter("out", [1, 1], F32, isOutput=True)
    pt_g = pt_e.rearrange("(g k p) c -> g p k c", g=2, p=P)  # [2][128, 2, 2048]

    with tile.TileContext(nc) as tc:
        with (
            tc.tile_pool(name="const", bufs=1) as const,
            tc.tile_pool(name="persist", bufs=1) as pers,
            tc.tile_pool(name="ptst", bufs=2) as ptst,
            tc.tile_pool(name="zst", bufs=2) as zstp,
            tc.tile_pool(name="psum", bufs=2, space="PSUM") as psp,
        ):
            ident = const.tile([P, P], F32, name="ident", tag="ident")
            make_identity(nc, ident[:])
            ln2_b = const.tile([P, 1], F32, name="ln2_b", tag="ln2_b")
            nc.gpsimd.memset(ln2_b[:], LN2)
            one_b = const.tile([P, 1], F32, name="one_b", tag="one_b")
            nc.gpsimd.memset(one_b[:], 1.0)
            ones16 = const.tile([P, 1], MM_DT, name="ones16", tag="ones16")
            nc.gpsimd.memset(ones16[:], 1.0)
            # zaug rows [x2_b; 1]: ones via memset, row 0 filled below
            zaug = pers.tile([2, BL], MM_DT, name="zaug", tag="zaug")
            nc.gpsimd.memset(zaug[:, :], 1.0)

            warm = const.tile([1, 1], F32, name="warm", tag="warm")
            nc.scalar.activation(warm[:], one_b[0:1, :], AF.Ln)

            # ---- pt: 2 grouped DMAs, first on sync, second on scalar ------
            ptg_tiles = []
            for g in range(2):
                ptg = ptst.tile([P, 2, C], F32, name=f"ptg{g}", tag="ptst")
                (nc.sync if g == 0 else nc.scalar).dma_start(out=ptg[:], in_=pt_g[g])
                ptg_tiles.append(ptg)

            # ---- zT bulk load: 8 DMAs on gpsimd queue, low halves first ---
            ztr = [
                pers.tile([P, BL], MM_DT, name=f"ztr{k}", tag=f"ztr{k}")
                for k in range(KC)
            ]
            zst_tiles = {}
            for half in range(2):
                hs = slice(half * 2048, (half + 1) * 2048)
                for k in range(KC):
                    zst = zstp.tile([P, 2048], F32, name=f"zt{k}_{half}", tag="zst")
                    nc.gpsimd.dma_start(out=zst[:], in_=zt_e[k * P : (k + 1) * P, hs])
                    zst_tiles[(k, half)] = zst
                    if half == 0:
                        nc.vector.tensor_copy(ztr[k][:, hs], zst[:])

            # ---- y2 / inv / paug / invb, piecewise over 4 c-chunks --------
            ptsq = [
                pers.tile([P, C], MM_DT, name=f"ptsq{k}", tag=f"ptsq{k}")
                for k in range(KC)
            ]
            for k in range(KC):
                nc.scalar.activation(
                    ptsq[k][:], ptg_tiles[k // 2][:, k % 2, :], AF.Square
                )
            paug = pers.tile([2, C], MM_DT, name="paug", tag="paug")
            invrow = pers.tile([1, C], F32, name="invrow", tag="invrow")
            y2i16 = pers.tile([1, C], MM_DT, name="y2i16", tag="y2i16")
            invb = pers.tile([P, C], MM_DT, name="invb", tag="invb")
            y2p = psp.tile([1, C], F32, name="y2p", tag="mm")
            for cc in range(4):
                cs = slice(cc * 512, (cc + 1) * 512)
                for k in range(KC):
                    nc.tensor.matmul(
                        y2p[0:1, cs], ones16[:], ptsq[k][:, cs],
                        start=(k == 0), stop=(k == KC - 1),
                    )
            omy = pers.tile([1, C], F32, name="omy", tag="omy")
            nc.vector.tensor_scalar(omy[:], y2p[:], -1.0, 1.0, ALU.mult, ALU.add)
            nc.vector.reciprocal(invrow[:], omy[:])
            nc.vector.tensor_copy(paug[0:1, :], invrow[:])
            nc.vector.tensor_scalar_add(y2i16[:], invrow[:], -1.0)
            nc.gpsimd.partition_broadcast(invb[:], paug[0:1, :])
            # y2 inv - 1 row to partition 1 of paug
            nc.sync.dma_start(out=paug[1:2, :], in_=y2i16[0:1, :])

            # ---- scaled protos: psc = pt * (-2 inv), piecewise ------------
            psc = [
                pers.tile([P, C], MM_DT, name=f"psc{k}", tag=f"psc{k}")
                for k in range(KC)
            ]
            for k in range(KC):
                for h in range(2):
                    hs = slice(h * 1024, (h + 1) * 1024)
                    nc.vector.scalar_tensor_tensor(
                        psc[k][:, hs],
                        ptg_tiles[k // 2][:, k % 2, hs],
                        -2.0,
                        invb[:, hs],
                        op0=ALU.mult,
                        op1=ALU.mult,
                    )

            # ---- remaining zT casts (high halves), after the psc chain ----
            for k in range(KC):
                nc.vector.tensor_copy(ztr[k][:, 2048:4096], zst_tiles[(k, 1)][:])

            # ---- x2 via squared zt chunks + ones matmul -------------------
            zsq_tiles = {}
            for half in range(2):
                hs = slice(half * 2048, (half + 1) * 2048)
                for k in range(KC):
                    zsq = pers.tile(
                        [P, 2048], MM_DT, name=f"zsq{k}_{half}", tag=f"zsq{k}_{half}"
                    )
                    nc.scalar.activation(zsq[:], ztr[k][:, hs], AF.Square)
                    zsq_tiles[(k, half)] = zsq

            def x2_half(half):
                x2p = psp.tile([1, 2048], F32, name=f"x2p{half}", tag="mm")
                for cc in range(4):
                    cs = slice(cc * 512, (cc + 1) * 512)
                    for k in range(KC):
                        nc.tensor.matmul(
                            x2p[0:1, cs], ones16[:],
                            zsq_tiles[(k, half)][:, cs],
                            start=(k == 0), stop=(k == KC - 1),
                        )
                nc.vector.tensor_copy(
                    zaug[0:1, half * 2048 : (half + 1) * 2048], x2p[:]
                )

            x2_half(0)

            mg_sb = const.tile([P, 1], F32, name="mg_sb", tag="mg_sb")
            nc.sync.dma_start(out=mg_sb[:], in_=mg_e[:, :])

            # ---- main loop -------------------------------------------------
            mcol = pers.tile([P, MT], F32, name="mcol", tag="mcol")
            for m in range(MT):
                if m == 8:
                    x2_half(1)
                ms = slice(m * P, (m + 1) * P)
                pm = psp.tile([P, C], F32, name=f"mm{m}", tag="mm")
                for k in range(KC):
                    for n in range(NT):
                        ns = slice(n * 512, (n + 1) * 512)
                        nc.tensor.matmul(
                            pm[:, ns],
                            ztr[k][:, ms],
                            psc[k][:, ns],
                            start=(k == 0),
                            stop=False,
                        )
                for n in range(NT):
                    ns = slice(n * 512, (n + 1) * 512)
                    nc.tensor.matmul(
                        pm[:, ns], zaug[:, ms], paug[:, ns], start=False, stop=True
                    )
                nc.vector.tensor_reduce(
                    mcol[:, m : m + 1], pm[:], axis=AX.X, op=ALU.min
                )

            # ---- x2 (zaug row 0) -> [128, 32] layout for the epilogue ----
            xT16 = pers.tile([MT, P], MM_DT, name="xT16", tag="xT16")
            nc.sync.dma_start(out=xT16[:, :], in_=zaug[0:1, :])
            ident16 = const.tile([MT, MT], MM_DT, name="ident16", tag="ident16")
            make_identity(nc, ident16[:])
            x2ps = psp.tile([P, MT], MM_DT, name="x2ps", tag="mm")
            nc.tensor.transpose(x2ps[:], xT16[:], ident16[:])
            x2c = pers.tile([P, MT], F32, name="x2c", tag="x2c")
            nc.vector.tensor_copy(x2c[:], x2ps[:])
            omx = pers.tile([P, MT], F32, name="omx", tag="omx")
            nc.vector.tensor_scalar(omx[:], x2c[:], -1.0, 1.0, ALU.mult, ALU.add)
            invx = pers.tile([P, MT], F32, name="invx", tag="invx")
            nc.vector.reciprocal(invx[:], omx[:])

            # ---- epilogue: dist = ln(arg + sqrt(arg^2-1)), loss sum -------
            ep = lambda nm: pers.tile([P, MT], F32, name=nm, tag=nm)
            mre = ep("mre")
            nc.vector.tensor_scalar_max(mre[:], mcol[:], 0.0)
            t = ep("t")
            nc.vector.tensor_tensor(t[:], mre[:], invx[:], op=ALU.mult)
            t2 = ep("t2")
            nc.vector.tensor_scalar_max(t2[:], t[:], EPS / 2)
            # arg = 1 + 2*t2; arg^2-1 = 4*t2*(t2+1); sqrt via exp(ln/2)
            u = ep("u")
            nc.vector.scalar_tensor_tensor(
                u[:], t2[:], 1.0, t2[:], op0=ALU.add, op1=ALU.mult
            )
            lnu = ep("lnu")
            nc.scalar.activation(lnu[:], u[:], AF.Ln)
            w = ep("w")
            nc.scalar.activation(w[:], lnu[:], AF.Exp, scale=0.5, bias=ln2_b[:])
            v = ep("v")
            nc.vector.scalar_tensor_tensor(
                v[:], t2[:], 2.0, w[:], op0=ALU.mult, op1=ALU.add
            )
            dd = ep("dd")
            nc.scalar.activation(dd[:], v[:], AF.Ln, bias=one_b[:])
            li = ep("li")
            nc.vector.tensor_scalar(
                li[:], dd[:], mg_sb[:], 0.0, ALU.subtract, ALU.min
            )
            lsum = pers.tile([P, 1], F32, name="lsum", tag="lsum")
            nc.vector.tensor_reduce(lsum[:], li[:], axis=AX.X, op=ALU.add)
            tot = pers.tile([1, 1], F32, name="tot", tag="tot")
            nc.gpsimd.tensor_reduce(tot[:], lsum[:], axis=AX.C, op=ALU.add)
            tots = pers.tile([1, 1], F32, name="tots", tag="tots")
            nc.vector.tensor_scalar_mul(tots[:], tot[:], -1.0 / B)
            nc.sync.dma_start(out=out_e[:, :], in_=tots[:])

    nc.compile()
    return nc


def _get_nc():
    if "nc" not in _NC_CACHE:
        _NC_CACHE["nc"] = _build_nc()
    return _NC_CACHE["nc"]


def _make_in_maps(z, p, marg):
    pt = np.ascontiguousarray(p.T)
    mg = np.full((P, 1), marg, np.float32)
    in_maps = []
    for i in range(NCORES):
        sh = z[i * BL : (i + 1) * BL]
        in_maps.append(
            {
                "zt": np.ascontiguousarray(sh.T),
                "pt": pt,
                "margin": mg,
            }
        )
    return in_maps


def _run(inputs, trace=False):
    z = np.asarray(inputs["z_mix"], np.float32)
    p = np.asarray(inputs["prototypes"], np.float32)
    marg = np.float32(np.asarray(inputs["repel_margin"]).reshape(-1)[0])
    nc = _get_nc()
    res = run_bass_kernel_spmd(
        nc, _make_in_maps(z, p, marg), core_ids=list(range(NCORES)), trace=trace
    )
    total = sum(float(r["out"][0, 0]) for r in res.results)
    return np.float32(total), res


def kernel(**inputs) -> np.ndarray:
    out, _ = _run(inputs, trace=False)
    return out
